# revision 36
# baseline (speedup 1.0000x reference)
"""Trainium2 Bass kernel for nn_MultiHeadedAttention_30210799960138.

Reference semantics (B=2, T=2048, E=2048, H=8 heads, MQA num_kv=1, D=256):
  q = x @ Wq ; k = x @ Wk ; v = x @ Wv          (biases are zeros)
  q -> reshape(B, H, T, D) (pure C-order reshape: Q[h,t,:] =
       q2d[256h + t//8, 256*(t%8) : 256*(t%8)+256])
  scores = (Q_h @ K^T) * sqrt(D); probs = softmax(scores)
  O_h = probs @ V ; final[t, 256h+d] = O_h[t, d] ; final @ Wo + bo

Sharding (8 cores): core c handles batch b = c // 4 and the query block
t in [512g, 512(g+1)), g = c % 4, for ALL 8 heads. Each core computes the
full K/V projections for its batch (duplicated 4x, unavoidable without
collectives), the Q projection for its 512 query rows, attention for all
heads on its query block, and the final output projection rows. The host
only places the 8 disjoint row-blocks (no partial sums).

Precision: the score path (x, Wq, Wk, Q^T, K^T, score matmuls) runs in
float32r - fp32 operands read at ~FP22 by the PE, which at moving dim
>= 256 runs at full bf16 rate (1 cycle/row), unlike true fp32 (4x).
The value path (V, probs, O, Wo) runs in bf16. Softmax uses a global
per-row max over 4 psum quarters, exp on the scalar engine with the
sqrt(D)=16 scale folded into the activation scale, and 1/Z applied as a
single bf16 DVE multiply on the probability tile. CPU simulation of this
scheme measures rel err ~2.7e-3 vs the fp32 reference (gate is 2e-2);
bf16 projections would fail (5.6e-2) because score std is ~256 and the
softmax is near-argmax.
"""

import numpy as np

B, T, E = 2, 2048, 2048
H_TOT, D = 8, 256
P = 128
EC = E // P      # 16 contraction chunks
QB = 512         # queries per core

_CACHED = None   # compiled Bacc program
LAST_RESULT = None  # BassKernelResults of the most recent run (for test.py)


def _build_bass():
    import concourse.bacc as bacc
    import concourse.mybir as mybir
    import concourse.tile as tile
    from concourse.masks import make_identity
    from contextlib import ExitStack

    F32 = mybir.dt.float32
    F32R = mybir.dt.float32r
    BF16 = mybir.dt.bfloat16
    EXP = mybir.ActivationFunctionType.Exp
    AX = mybir.AxisListType.X

    nc = bacc.Bacc("TRN2", target_bir_lowering=False, debug=False)

    def din(name, shape, dt):
        return nc.dram_tensor(name, shape, dt, kind="ExternalInput").ap()

    xT = din("xT", [E, T], F32)            # x^T (full batch) for K/V proj
    xTq = din("xTq", [E, QB], F32)         # packed q-row cols: j = 64h + w
    Wq = din("Wq", [E, E], F32)
    Wk = din("Wk", [E, D], F32)
    Wv = din("Wv", [E, D], F32)
    Wo = din("Wo", [E, E], BF16)
    out = nc.dram_tensor("out", [QB, E], F32, kind="ExternalOutput").ap()

    def r3(ap):  # [E, N] -> [128, EC, N]
        return ap.rearrange("(ko p) t -> p ko t", p=P)

    xT_r, xTq_r = r3(xT), r3(xTq)
    Wq_r, Wk_r, Wv_r = r3(Wq), r3(Wk), r3(Wv)
    Wo_r = Wo.rearrange("(cc p) e -> p cc e", p=P)       # [128, 16, 2048]

    with tile.TileContext(nc) as tc:
        with ExitStack() as ctx:
            persist = ctx.enter_context(tc.tile_pool(name="persist", bufs=1))

            # ---- persistent tensors ----
            KT = persist.tile([P, 2, T], F32R)           # K^T, d on partitions
            V = persist.tile([P, EC, D], BF16)           # V, t on partitions
            # Q^T packed: [d_local, dh, head, tl]  (tl = local query idx)
            QT = persist.tile([P, 2, H_TOT, QB], F32R)
            OT = persist.tile([P, EC, QB], BF16)         # O^T, c on partitions
            ident = persist.tile([P, P], BF16)
            make_identity(nc, ident)

            # ======== Phase A+B: K/V and Q projections, interleaved ========
            # Rounds of (K, V for one 512-token block, then 4 Q out-chunks)
            # keep the tensor engine fed from the first 6 MB of DMA onward
            # instead of serializing the 36 MB of projection inputs.
            with ExitStack() as actx:
                wkv = actx.enter_context(tc.tile_pool(name="wkv", bufs=1))
                xs = actx.enter_context(tc.tile_pool(name="xs", bufs=1))
                xqp = actx.enter_context(tc.tile_pool(name="xqp", bufs=1))
                wqs = actx.enter_context(tc.tile_pool(name="wqs", bufs=2))
                ps_v = actx.enter_context(
                    tc.tile_pool(name="ps_v", bufs=2, space="PSUM"))
                ps_k = actx.enter_context(
                    tc.tile_pool(name="ps_k", bufs=2, space="PSUM"))
                ps_q = actx.enter_context(
                    tc.tile_pool(name="ps_q", bufs=3, space="PSUM"))

                # DMA priority order: first half token block, Wk, second
                # half, Wv, then the Q-side inputs; round 0's K runs in
                # 256-col half chains so it starts ~6us earlier.
                xblk = xs.tile([P, EC, QB], F32R, tag="xblk")
                nc.sync.dma_start(xblk[:, :, 0:256],
                                  xT_r[:, :, 0:256].bitcast(F32R))
                wk_sb = wkv.tile([P, EC, D], F32R)
                nc.sync.dma_start(wk_sb[:, :, 0:P],
                                  Wk_r[:, :, 0:P].bitcast(F32R))
                nc.sync.dma_start(wk_sb[:, :, P:D],
                                  Wk_r[:, :, P:D].bitcast(F32R))
                nc.sync.dma_start(xblk[:, :, 256:QB],
                                  xT_r[:, :, 256:QB].bitcast(F32R))
                wv_sb = wkv.tile([P, EC, D], F32R)
                nc.sync.dma_start(wv_sb, Wv_r.bitcast(F32R))
                # xtq in halves with the first Wq pair between them, so
                # round 0's Q half-chains start ~7us earlier.
                xtq_sb = xqp.tile([P, EC, QB], F32R)
                nc.sync.dma_start(xtq_sb[:, :, 0:256],
                                  xTq_r[:, :, 0:256].bitcast(F32R))
                wq_first = wqs.tile([P, EC, 2 * P], F32R, tag="wq",
                                    name="wq_first")
                nc.sync.dma_start(wq_first, Wq_r[:, :, 0:2 * P].bitcast(F32R))
                nc.sync.dma_start(xtq_sb[:, :, 256:QB],
                                  xTq_r[:, :, 256:QB].bitcast(F32R))

                for r in range(4):
                    if r > 0:
                        sl = slice(r * QB, (r + 1) * QB)
                        xblk = xs.tile([P, EC, QB], F32R, tag="xblk")
                        nc.sync.dma_start(xblk, xT_r[:, :, sl].bitcast(F32R))
                    # K^T cols for this token block
                    halves = ((0, 256), (256, QB)) if r == 0 else ((0, QB),)
                    for dh in range(2):
                        kp = ps_k.tile([P, QB], F32, tag="kp")
                        for lo, hi in halves:
                            for ec in range(EC):
                                nc.tensor.matmul(
                                    kp[:, lo:hi],
                                    lhsT=wk_sb[:, ec, dh * P:(dh + 1) * P],
                                    rhs=xblk[:, ec, lo:hi],
                                    start=(ec == 0), stop=(ec == EC - 1))
                        nc.any.tensor_copy(out=KT[:, dh, r * QB:(r + 1) * QB],
                                           in_=kp)
                    # V rows for this token block
                    for j in range(4):
                        vp = ps_v.tile([P, D], F32, tag="vp")
                        for ec in range(EC):
                            nc.tensor.matmul(
                                vp,
                                lhsT=xblk[:, ec, j * P:(j + 1) * P],
                                rhs=wv_sb[:, ec, :],
                                start=(ec == 0), stop=(ec == EC - 1))
                        nc.any.tensor_copy(out=V[:, 4 * r + j, :], in_=vp)
                    # Q^T out-chunks 4r .. 4r+3
                    for qh in range(2):
                        if r == 0 and qh == 0:
                            wq_blk = wq_first
                        else:
                            wq_blk = wqs.tile([P, EC, 2 * P], F32R, tag="wq")
                            base = (4 * r + 2 * qh) * P
                            nc.sync.dma_start(
                                wq_blk,
                                Wq_r[:, :, base:base + 2 * P].bitcast(F32R))
                        for qi in range(2):
                            qc = 4 * r + 2 * qh + qi
                            c, dh = qc // 2, qc % 2
                            ps = ps_q.tile([P, QB], F32, tag="pq")
                            for lo, hi in halves:
                                for ec in range(EC):
                                    nc.tensor.matmul(
                                        ps[:, lo:hi],
                                        lhsT=wq_blk[:, ec,
                                                    qi * P:(qi + 1) * P],
                                        rhs=xtq_sb[:, ec, lo:hi],
                                        start=(ec == 0), stop=(ec == EC - 1))
                            # QT[p, dh, h, 8w + c] = ps[p, 64h + w]
                            dst = QT[:, dh].rearrange(
                                "p h (w c8) -> p h w c8", c8=8)[:, :, :, c]
                            src = ps.rearrange("p (h w) -> p h w", h=H_TOT)
                            nc.any.tensor_copy(out=dst, in_=src)

            # ======== Phase C: attention ========
            # Wo lands in the space freed by the A/B pools; it stays live
            # through phase D.
            wop = ctx.enter_context(tc.tile_pool(name="wop", bufs=1))
            wo_sb = wop.tile([P, EC, E], BF16)
            with ExitStack() as cctx:
                ppool = cctx.enter_context(tc.tile_pool(name="ppool", bufs=4))
                ptpool = cctx.enter_context(tc.tile_pool(name="ptpool", bufs=2))
                stat = cctx.enter_context(tc.tile_pool(name="stat", bufs=12))
                ps_s = cctx.enter_context(
                    tc.tile_pool(name="ps_s", bufs=5, space="PSUM"))
                ps_t = cctx.enter_context(
                    tc.tile_pool(name="ps_t", bufs=2, space="PSUM"))
                ps_pv = cctx.enter_context(
                    tc.tile_pool(name="ps_pv", bufs=1, space="PSUM"))

                NQ = 4
                QW = T // NQ     # 512 keys per quarter

                pt_tiles = {}

                def emit_scores(h, m):
                    """Scores + softmax for one 128-query chunk; returns
                    the probability tile. Global-row-max form: quarter
                    psums are held until all stops land, which keeps the
                    softmax reads off the matmuls' backs (concurrent psum
                    traffic measurably stretches PE instructions)."""
                    s_ps = [ps_s.tile([P, QW], F32, tag="s",
                                      name=f"s_{h}_{m}_{qi}")
                            for qi in range(NQ)]
                    for dh in range(2):
                        for qi in range(NQ):
                            nc.tensor.matmul(
                                s_ps[qi],
                                lhsT=QT[:, dh, h, m * P:(m + 1) * P],
                                rhs=KT[:, dh, qi * QW:(qi + 1) * QW],
                                start=(dh == 0), stop=(dh == 1))
                    nmq = stat.tile([P, NQ], F32, tag="nmq")
                    for qi in range(NQ):
                        nc.vector.reduce_max(
                            nmq[:, qi:qi + 1], s_ps[qi], axis=AX,
                            negate=True)
                    nmM = stat.tile([P, 1], F32, tag="nmM")
                    nc.vector.tensor_reduce(
                        nmM, nmq, axis=AX, op=mybir.AluOpType.min)
                    bias16 = stat.tile([P, 1], F32, tag="b16")
                    nc.vector.tensor_scalar_mul(bias16, nmM, 16.0)
                    # p = exp(16*s - 16*M) / Z, bf16
                    p_sb = ppool.tile([P, T], BF16, tag="p")
                    smq = stat.tile([P, NQ], F32, tag="smq")
                    for qi in range(NQ):
                        nc.scalar.activation(
                            out=p_sb[:, qi * QW:(qi + 1) * QW],
                            in_=s_ps[qi], func=EXP,
                            bias=bias16, scale=16.0,
                            accum_out=smq[:, qi:qi + 1])
                    z = stat.tile([P, 1], F32, tag="z")
                    nc.vector.reduce_sum(z, smq, axis=AX)
                    zrec = stat.tile([P, 1], F32, tag="zr")
                    nc.vector.reciprocal(zrec, z)
                    nc.vector.tensor_scalar_mul(p_sb, p_sb, zrec)
                    return p_sb

                def emit_transposes(h, m, p_sb):
                    """Transpose chunk m into PT."""
                    pt_sb = pt_tiles[h]
                    for g in range(4):
                        tp = ps_t.tile([P, 4 * P], BF16, tag="tp")
                        for j in range(4):
                            nc.tensor.transpose(
                                tp[:, j * P:(j + 1) * P],
                                p_sb[:, (4 * g + j) * P:
                                     (4 * g + j + 1) * P],
                                ident)
                        nc.any.tensor_copy(
                            out=pt_sb[:, 4 * g:4 * (g + 1),
                                      m * P:(m + 1) * P],
                            in_=tp.rearrange("p (a b) -> p a b", a=4))

                def emit_pv(h, dh):
                    pt_sb = pt_tiles[h]
                    op = ps_pv.tile([P, QB], F32, tag="op")
                    for kc in range(EC):
                        nc.tensor.matmul(
                            op,
                            lhsT=V[:, kc, dh * P:(dh + 1) * P],
                            rhs=pt_sb[:, kc, :],
                            start=(kc == 0), stop=(kc == EC - 1))
                    nc.any.tensor_copy(out=OT[:, 2 * h + dh, :], in_=op)

                # 2-deep software pipeline: chunk m's transposes are
                # emitted after chunk m+2's scores, so by the time the
                # tensor engine reaches them the softmax chain (vector +
                # scalar) has long finished and nothing stalls. The PV
                # half-chains are spread one per unit so every unit has
                # enough tensor filler to cover the softmax chain.
                pending = []
                pv_q = []

                def run_tail():
                    hh, mm, pp = pending.pop(0)
                    emit_transposes(hh, mm, pp)
                    if mm == 3:
                        pv_q.extend([(hh, 0), (hh, 1)])

                for h in range(H_TOT):
                    if h == 1:
                        # Wo queues behind all projection inputs; attention
                        # leaves the DMA engines otherwise idle.
                        nc.sync.dma_start(wo_sb, Wo_r)
                    pt_tiles[h] = ptpool.tile([P, EC, QB], BF16, tag="pt",
                                              name=f"pt_{h}")
                    for m in range(4):
                        p_sb = emit_scores(h, m)
                        if len(pending) >= 2:
                            run_tail()
                        pending.append((h, m, p_sb))
                        if pv_q:
                            emit_pv(*pv_q.pop(0))
                while pending:
                    run_tail()
                while pv_q:
                    emit_pv(*pv_q.pop(0))

            # ======== Phase D: output projection ========
            with ExitStack() as dctx:
                obuf = dctx.enter_context(tc.tile_pool(name="obuf", bufs=2))
                ps_f = dctx.enter_context(
                    tc.tile_pool(name="ps_f", bufs=2, space="PSUM"))

                for qc in range(4):
                    o_sb = obuf.tile([P, E], F32, tag="o")
                    for eb in range(4):
                        fp = ps_f.tile([P, 512], F32, tag="fp")
                        for cc in range(EC):
                            nc.tensor.matmul(
                                fp,
                                lhsT=OT[:, cc, qc * P:(qc + 1) * P],
                                rhs=wo_sb[:, cc, eb * 512:(eb + 1) * 512],
                                start=(cc == 0), stop=(cc == EC - 1))
                        nc.any.tensor_copy(
                            out=o_sb[:, eb * 512:(eb + 1) * 512], in_=fp)
                        # per-block writeout so the final DMA trails the
                        # last psum copy by ~1us instead of a full row.
                        nc.sync.dma_start(
                            out[qc * P:(qc + 1) * P,
                                eb * 512:(eb + 1) * 512],
                            o_sb[:, eb * 512:(eb + 1) * 512])

    nc.compile()
    return nc


def _get_program():
    global _CACHED
    if _CACHED is None:
        _CACHED = _build_bass()
    return _CACHED


def kernel(x, attention_mask, Wq, bq, Wk, bk, Wv, bv, Wo, bo):
    import ml_dtypes
    from concourse import bass_utils

    x = np.asarray(x, dtype=np.float32)
    Wq = np.ascontiguousarray(np.asarray(Wq, dtype=np.float32))
    Wk = np.ascontiguousarray(np.asarray(Wk, dtype=np.float32))
    Wv = np.ascontiguousarray(np.asarray(Wv, dtype=np.float32))
    Wo_bf = np.asarray(Wo, dtype=np.float32).astype(ml_dtypes.bfloat16)
    bo = np.asarray(bo, dtype=np.float32)

    nc = _get_program()

    xTs = [np.ascontiguousarray(x[b].T) for b in range(B)]

    in_maps = []
    for c in range(8):
        b, g = c // 4, c % 4
        qcols = (256 * np.arange(8)[:, None]
                 + 64 * g + np.arange(64)[None, :]).reshape(-1)
        in_maps.append({
            "xT": xTs[b],
            "xTq": np.ascontiguousarray(xTs[b][:, qcols]),
            "Wq": Wq,
            "Wk": Wk,
            "Wv": Wv,
            "Wo": Wo_bf,
        })

    res = bass_utils.run_bass_kernel_spmd(nc, in_maps, core_ids=list(range(8)))
    global LAST_RESULT
    LAST_RESULT = res

    final = np.empty((B, T, E), dtype=np.float32)
    for c in range(8):
        b, g = c // 4, c % 4
        final[b, QB * g:QB * (g + 1), :] = res.results[c]["out"]
    final += bo[None, None, :]
    return final
